# revision 72
# baseline (speedup 1.0000x reference)
"""Trainium2 Bass kernel for EfficientAttention (linear attention block), v2.

Computation (per batch b, head h):
    qkv = x @ w_qkv.T (+ b_qkv)
    q = softmax(q, axis=head_dim) * head_dim**-0.5
    k = softmax(k, axis=seqlen)
    kv[d,e] = sum_s k[s,d] v[s,e]          (per-head 64x64 state)
    out[s,e] = sum_d q[s,d] kv[d,e]
    y = out @ w_proj.T (+ b_proj)

Sharding: 8 cores = (batch b = c//2, seq half = c%2); 2048 tokens per core,
all 16 heads. Cross-core coupling is only the kv state and the k-softmax
denominator Z -> one small AllReduce (pairs of cores) of [129, 1024] fp32.

v2 design vs v1 (523us):
- all matmuls in bf16 (fp32 "HIGH" PE mode streams ~1.9 Grow/s and is
  power-throttled; quantization error ~0.1% per tensor, budget 2e-2)
- M-fold: M[hd,j] = sum_e kv_norm[d,e] w_proj.T[e,j] computed once after
  the collective (8 matmuls), replacing the separate q@kv stage and its
  PSUM->SBUF copies; out-proj contracts q_T directly against M.
- kv state accumulated in PSUM across 4-tb blocks (pair-major group order
  to avoid interleaved accumulation groups per bank), folded to SBUF by
  DVE once per block.
- q normalization: one DVE broadcast-multiply (stride-0 AP) instead of 16
  scalar-engine ops per token block (scalar op overhead ~0.45us each).
- eq -> qT transposes moved off the PE onto the DMA XBAR (8x [128,128]
  bf16 transposes per token block, on otherwise-idle queues).
- x loaded once (bf16), kept in SBUF for both the k/v and q sweeps.
- y stores via one ACT copy [128,1024] + one DMA per token block.
"""

import sys

sys.path.insert(0, "/opt/trn_rl_repo")

import numpy as np
import ml_dtypes

import concourse.bacc as bacc
import concourse.bass as bass
import concourse.tile as tile
from concourse import mybir
from concourse import bass_utils

F32 = mybir.dt.float32
BF16 = mybir.dt.bfloat16

D = 1024          # model dim (= qkv contraction dim)
T = 2048          # tokens per core (one batch element's half sequence)
NH = 16           # heads
HD = 64           # head dim
NPAIR = 8         # head pairs (2 heads / 128 partitions)
KC = D // 128     # contraction chunks of 128
TB = T // 128     # token blocks of 128
SCALE = HD ** -0.5

N_CORES = 8


def bcast_part(ap, n=128):
    """View a [1, N] AP as [n, N] with partition step 0 (DMA broadcast)."""
    return bass.AP(tensor=ap.tensor, offset=ap.offset,
                   ap=[[0, n]] + list(ap.ap[1:]))


def build_program(with_bias=False):
    nc = bacc.Bacc("TRN2", target_bir_lowering=False, num_devices=N_CORES)

    xt = nc.dram_tensor("xt", [D, T], BF16, kind="ExternalInput")    # x chunk, transposed
    wq = nc.dram_tensor("wq", [D, D], BF16, kind="ExternalInput")    # w_q.T
    wk = nc.dram_tensor("wk", [D, D], BF16, kind="ExternalInput")    # w_k.T
    wv = nc.dram_tensor("wv", [D, D], BF16, kind="ExternalInput")    # w_v.T
    wp = nc.dram_tensor("wp", [D, D], BF16, kind="ExternalInput")    # w_proj.T
    identd = nc.dram_tensor("identd", [128, 128], BF16, kind="ExternalInput")
    if with_bias:
        ebq = nc.dram_tensor("ebq", [D], F32, kind="ExternalInput")  # exp(b_q)
        by = nc.dram_tensor("by", [D], F32, kind="ExternalInput")    # folded out bias
    y = nc.dram_tensor("y", [T, D], F32, kind="ExternalOutput")

    xt_v = xt.rearrange("(kc p) t -> p kc t", p=128)
    wq_v = wq.rearrange("(kc p) f -> p kc f", p=128)
    wk_v = wk.rearrange("(kc p) f -> p kc f", p=128)
    wv_v = wv.rearrange("(kc p) f -> p kc f", p=128)
    wp_v = wp.rearrange("(kc p) f -> p kc f", p=128)

    with tile.TileContext(nc) as tc:
        with (
            tc.tile_pool(name="const", bufs=1) as const,
            tc.tile_pool(name="wpool", bufs=1) as wpool,
            tc.tile_pool(name="xpool", bufs=1) as xpool,
            tc.tile_pool(name="ekv", bufs=4) as ekv,
            tc.tile_pool(name="accp", bufs=1) as accp,
            tc.tile_pool(name="qpool", bufs=2) as qpool,
            tc.tile_pool(name="qtp", bufs=1) as qtp,
            tc.tile_pool(name="mpool", bufs=1) as mpool,
            tc.tile_pool(name="ytp", bufs=2) as ytp,
            tc.tile_pool(name="dram", bufs=1, space="DRAM") as dram,
        ):
            # x for the first token blocks and the k weights go first, split
            # across the two hwdge queues, so the first k-projection can
            # start as early as possible and is never starved of wk chunks.
            xall = xpool.tile([128, KC, T], BF16, tag="xall")
            nc.sync.dma_start(xall[:, :, 0:128], xt_v[:, :, 0:128])
            nc.scalar.dma_start(xall[:, :, 128:256], xt_v[:, :, 128:256])

            ident_sb = const.tile([128, 128], BF16, tag="ident")
            nc.gpsimd.dma_start(ident_sb, identd[:])
            if with_bias:
                ebq_sb = const.tile([128, D], F32, tag="ebq")
                nc.gpsimd.dma_start(ebq_sb, bcast_part(ebq[:].unsqueeze(0)))
                by_sb = const.tile([128, D], F32, tag="by")
                nc.gpsimd.dma_start(by_sb, bcast_part(by[:].unsqueeze(0)))

            # weights: per-chunk tiles so matmuls can start before the full
            # matrix lands. k/v weights first (phase 1), then q, then proj.
            wk_sb = [wpool.tile([128, D], BF16, tag=f"wk{kc}", name=f"wk{kc}")
                     for kc in range(KC)]
            wv_sb = [wpool.tile([128, D], BF16, tag=f"wv{kc}", name=f"wv{kc}")
                     for kc in range(KC)]
            wq_sb = [wpool.tile([128, D], BF16, tag=f"wq{kc}", name=f"wq{kc}")
                     for kc in range(KC)]
            wp_sb = [wpool.tile([128, D], BF16, tag=f"wp{kc}", name=f"wp{kc}")
                     for kc in range(KC)]
            for kc in range(KC):
                keng = nc.sync if kc % 2 == 0 else nc.scalar
                keng.dma_start(wk_sb[kc], wk_v[:, kc, :])
                nc.gpsimd.dma_start(wv_sb[kc], wv_v[:, kc, :])
            for kc in range(KC):
                nc.gpsimd.dma_start(wq_sb[kc], wq_v[:, kc, :])
                nc.gpsimd.dma_start(wp_sb[kc], wp_v[:, kc, :])

            kvacc = accp.tile([128, D], F32, tag="kvacc")
            zacc = accp.tile([128, KC], F32, tag="zacc")
            qtall = qtp.tile([128, KC, T], BF16, tag="qtall")

            # ---- Phase 1: k/v projections, exp(k), kv state + Z ----
            # kv is computed d-major (lhsT = ek pair), with a ones column
            # appended to v so each pair's matmul also produces its Z slice:
            # out[:, p, 0:128] = kv_p[d, e], out[:, p, 128] = Z[128p + d].
            # PSUM budget (8 banks): proj tag [128,1024] x2 bufs = 4 banks,
            # kv [128, 8, 256] = 4 banks (2 pairs per bank, groups ordered
            # pair-major so banks never see interleaved accumulation groups
            # -- interleaving corrupts, measured on HW).
            psum = tc.alloc_tile_pool(name="psum1", bufs=1, space="PSUM")
            kvps = psum.tile([128, NPAIR, 256], F32, tag="kv", bufs=1)
            vv_bufs = [ekv.tile([128, NPAIR, 132], BF16, tag=f"vv{j}",
                                name=f"vv{j}", bufs=1) for j in range(4)]
            for j in range(4):
                nc.gpsimd.memset(vv_bufs[j][:, :, 128:129], 1.0)
            eks = []
            for tb in range(TB):
                tsl = slice(tb * 128, (tb + 1) * 128)
                if tb >= 2:
                    eng = nc.sync if tb % 2 == 0 else nc.gpsimd
                    eng.dma_start(xall[:, :, tsl], xt_v[:, :, tsl])
                kps = psum.tile([128, D], F32, tag="proj", name="kps", bufs=2)
                for half in range(2):
                    sl = slice(half * 512, (half + 1) * 512)
                    for kc in range(KC):
                        nc.tensor.matmul(kps[:, sl], xall[:, kc, tsl],
                                         wk_sb[kc][:, sl],
                                         start=(kc == 0), stop=(kc == KC - 1))
                ek = ekv.tile([128, D], BF16, tag="ek")
                nc.scalar.activation(ek, kps, mybir.ActivationFunctionType.Exp)
                vps = psum.tile([128, D], F32, tag="proj", name="vps", bufs=2)
                for half in range(2):
                    sl = slice(half * 512, (half + 1) * 512)
                    for kc in range(KC):
                        nc.tensor.matmul(vps[:, sl], xall[:, kc, tsl],
                                         wv_sb[kc][:, sl],
                                         start=(kc == 0), stop=(kc == KC - 1))
                vv = vv_bufs[tb % 4]
                nc.scalar.copy(vv[:, :, 0:128],
                               vps[:].rearrange("p (g e) -> p g e", e=128))
                eks.append(ek)
                if tb % 4 == 3:
                    for p in range(NPAIR):
                        csl = slice(p * 128, (p + 1) * 128)
                        for j in range(4):
                            nc.tensor.matmul(kvps[:, p, 0:129],
                                             eks[j][:, csl],
                                             vv_bufs[j][:, p, 0:129],
                                             start=(j == 0), stop=(j == 3))
                    if tb == 3:
                        nc.vector.tensor_copy(kvacc[:].rearrange("p (g e) -> p g e", e=128),
                                              kvps[:, :, 0:128])
                        nc.vector.tensor_copy(zacc, kvps[:, :, 128])
                    else:
                        nc.vector.tensor_add(kvacc[:].rearrange("p (g e) -> p g e", e=128),
                                             kvacc[:].rearrange("p (g e) -> p g e", e=128),
                                             kvps[:, :, 0:128])
                        nc.vector.tensor_add(zacc, zacc, kvps[:, :, 128])
                    eks = []
            psum.release()

            # ---- AllReduce (kv | Z) across the sequence pair ----
            # Compact to the meaningful head-diagonal blocks in bf16
            # ([128, 520] = 133KB instead of 528KB): the collective lands
            # ~4x sooner, deterministically beating the point where the DVE
            # stream reaches its collective-gated ops (otherwise a ~10us
            # head-of-line PE stall appears, depending on scheduler luck).
            stage = accp.tile([128, 520], BF16, tag="stage")
            kvacc_v = kvacc[:].rearrange("p (g e) -> p g e", e=128)
            stage_v = stage[:, 0:512].rearrange("p (g e) -> p g e", e=64)
            nc.vector.tensor_copy(stage_v[0:64], kvacc_v[0:64, :, 0:64])
            nc.vector.tensor_copy(stage_v[64:128], kvacc_v[64:128, :, 64:128])
            nc.vector.tensor_copy(stage[:, 512:520], zacc)
            cin = dram.tile([128, 520], BF16, tag="cin")
            cout = dram.tile([128, 520], BF16, tag="cout")
            nc.sync.dma_start(cin, stage)
            nc.gpsimd.collective_compute(
                "AllReduce", mybir.AluOpType.add,
                replica_groups=[[0, 1], [2, 3], [4, 5], [6, 7]],
                ins=[cin[:].opt()], outs=[cout[:].opt()])
            kvred = accp.tile([128, 520], BF16, tag="kvred")
            nc.sync.dma_start(kvred, cout)

            # ---- Phase 2: q sweep (overlaps the collective) ----
            # PSUM: proj 2x2 banks, tp 2x1, ktp 1, mk 1 = 8.
            psum = tc.alloc_tile_pool(name="psum2", bufs=1, space="PSUM")
            pend = []  # (eq, tp, tsl) queue; transposes run with a 2-tb lag
            for tb in range(TB + 2):
                lag = pend.pop(0) if len(pend) >= 2 or tb >= TB else None
                if tb < TB:
                    tsl = slice(tb * 128, (tb + 1) * 128)
                    qps = psum.tile([128, D], F32, tag="proj", name="qps", bufs=2)
                    for half in range(2):
                        sl = slice(half * 512, (half + 1) * 512)
                        for kc in range(KC):
                            nc.tensor.matmul(qps[:, sl], xall[:, kc, tsl],
                                             wq_sb[kc][:, sl],
                                             start=(kc == 0), stop=(kc == KC - 1))
                            # interleave tb-2's transposes one at a time
                            # between 512-free matmuls so each 128-row eq
                            # weight load hides under a 216ns matmul (a
                            # back-to-back burst stalls on LDW; a 1-tb lag
                            # would stall on the DVE normalize instead).
                            # Single-shot writes to the tp bank are safe
                            # inside qps's open group: per-bank accumulation
                            # stays sequential.
                            if lag is not None and kc % 2 == 1:
                                p = 4 * half + kc // 2
                                nc.tensor.transpose(
                                    lag[1][:, p, :],
                                    lag[0][:, p * 128:(p + 1) * 128], ident_sb)
                elif lag is not None:
                    # flush: remaining transposes as plain bursts
                    for p in range(NPAIR):
                        nc.tensor.transpose(lag[1][:, p, :],
                                            lag[0][:, p * 128:(p + 1) * 128],
                                            ident_sb)
                if lag is not None:
                    nc.scalar.copy(qtall[:, :, lag[2]], lag[1])
                if tb >= TB:
                    continue
                eq = qpool.tile([128, D], BF16, tag="eq", bufs=3)
                nc.scalar.activation(eq, qps, mybir.ActivationFunctionType.Exp)
                if with_bias:
                    nc.vector.tensor_mul(eq, eq, ebq_sb)
                sums = qpool.tile([128, NH], F32, tag="sums")
                nc.vector.reduce_sum(sums, eq[:].rearrange("p (h e) -> p h e", e=HD),
                                     axis=mybir.AxisListType.X)
                rfac = qpool.tile([128, NH], F32, tag="rfac")
                nc.vector.reciprocal(rfac, sums)
                rfs = qpool.tile([128, NH], BF16, tag="rfs")
                nc.vector.tensor_scalar_mul(rfs, rfac, SCALE)
                # normalize: eq[p, h, e] *= rfs[p, h] via stride-0 broadcast
                rfs_ap = rfs[:]
                rfs_b = bass.AP(tensor=rfs_ap.tensor, offset=rfs_ap.offset,
                                ap=list(rfs_ap.ap) + [[0, HD]])
                eq_v = eq[:].rearrange("p (h e) -> p h e", e=HD)
                nc.vector.tensor_tensor(eq_v, eq_v, rfs_b, op=mybir.AluOpType.mult)
                tp = psum.tile([128, NPAIR, 128], BF16, tag="tp", name="tp", bufs=2)
                pend.append((eq, tp, tsl))

            # ---- unpack kv (d-major, cross-head blocks zero), transpose
            # pairs on the PE, fold with w_proj into M; 1/Z applied as M's
            # per-partition row scale ----
            rzs = accp.tile([128, KC], F32, tag="rzs")
            nc.vector.reciprocal(rzs, kvred[:, 512:520])
            kvsb = accp.tile([128, D], BF16, tag="kvsb")
            nc.gpsimd.memset(kvsb, 0.0)
            kvsb_v = kvsb[:].rearrange("p (g e) -> p g e", e=128)
            kvred_v = kvred[:, 0:512].rearrange("p (g e) -> p g e", e=64)
            nc.gpsimd.tensor_copy(kvsb_v[0:64, :, 0:64], kvred_v[0:64])
            nc.gpsimd.tensor_copy(kvsb_v[64:128, :, 64:128], kvred_v[64:128])
            ktp = psum.tile([128, NPAIR, 128], BF16, tag="ktp", name="ktp", bufs=1)
            for p in range(NPAIR):
                nc.tensor.transpose(ktp[:, p, :], kvsb[:, p * 128:(p + 1) * 128],
                                    ident_sb)
            kvt = accp.tile([128, D], BF16, tag="kvt")
            nc.vector.tensor_copy(kvt[:].rearrange("p (g e) -> p g e", e=128), ktp)
            m_sb = mpool.tile([128, KC, D], BF16, tag="m_sb")
            for p in range(NPAIR):
                for half in range(2):
                    sl = slice(half * 512, (half + 1) * 512)
                    # alternate the two spare 1-bank slots so the matmuls
                    # double-buffer against the ACT drains
                    mtag = "mk" if (2 * p + half) % 2 == 0 else "ktp"
                    mps = psum.tile([128, 512], F32, tag=mtag, name="mps", bufs=1)
                    nc.tensor.matmul(mps, kvt[:, p * 128:(p + 1) * 128],
                                     wp_sb[p][:, sl], start=True, stop=True)
                    # M_p rows are d-local of pair p: scale by 1/Z[128p + r]
                    nc.scalar.mul(m_sb[:, p, sl], mps, rzs[:, p:p + 1])

            # ---- Phase 4: out projection y = qT.T @ M ----
            # shares psum2's proj rotation: no pool boundary, so the first
            # y matmuls can slot in as soon as the rotation frees up.
            for tb in range(TB):
                tsl = slice(tb * 128, (tb + 1) * 128)
                yps = psum.tile([128, D], F32, tag="proj", name="yps", bufs=2)
                for half in range(2):
                    sl = slice(half * 512, (half + 1) * 512)
                    for c in range(KC):
                        nc.tensor.matmul(yps[:, sl], qtall[:, c, tsl],
                                         m_sb[:, c, sl],
                                         start=(c == 0), stop=(c == KC - 1))
                yt = ytp.tile([128, D], F32, tag="yt")
                if with_bias:
                    nc.vector.tensor_add(yt, yps, by_sb)
                    nc.sync.dma_start(y[tsl, :], yt)
                elif tb == TB - 1:
                    # split the last tile so the copy and store overlap and
                    # the final DMA is half as long (teardown tail)
                    nc.scalar.copy(yt[:, 0:512], yps[:, 0:512])
                    nc.sync.dma_start(y[tsl, 0:512], yt[:, 0:512])
                    nc.scalar.copy(yt[:, 512:1024], yps[:, 512:1024])
                    nc.sync.dma_start(y[tsl, 512:1024], yt[:, 512:1024])
                else:
                    nc.scalar.copy(yt, yps)
                    nc.sync.dma_start(y[tsl, :], yt)
            psum.release()

    nc.compile()
    return nc


_NC = {}


def _get_nc(with_bias=False):
    if with_bias not in _NC:
        _NC[with_bias] = build_program(with_bias=with_bias)
    return _NC[with_bias]


def kernel(x, w_qkv, b_qkv, w_proj, b_proj):
    x = np.asarray(x, dtype=np.float32)
    w_qkv = np.asarray(w_qkv, dtype=np.float32)
    b_qkv = np.asarray(b_qkv, dtype=np.float32)
    w_proj = np.asarray(w_proj, dtype=np.float32)
    b_proj = np.asarray(b_proj, dtype=np.float32)

    bs, seqlen, dim = x.shape
    half = seqlen // 2
    bf = ml_dtypes.bfloat16

    wqm = np.ascontiguousarray(w_qkv[0:D].T.astype(bf))
    wkm = np.ascontiguousarray(w_qkv[D:2 * D].T.astype(bf))
    wvm = np.ascontiguousarray(w_qkv[2 * D:3 * D].T.astype(bf))
    wpm = np.ascontiguousarray(w_proj.T.astype(bf))
    bq, bv = b_qkv[0:D], b_qkv[2 * D:3 * D]

    ident = np.eye(128, dtype=bf)

    with_bias = bool(np.any(b_qkv)) or bool(np.any(b_proj))

    in_maps = []
    for c in range(N_CORES):
        b, s = divmod(c, 2)
        chunk = np.ascontiguousarray(x[b, s * half:(s + 1) * half, :].T.astype(bf))
        im = {"xt": chunk, "wq": wqm, "wk": wkm, "wv": wvm, "wp": wpm,
              "identd": ident}
        if with_bias:
            im["ebq"] = np.exp(bq).astype(np.float32)
            im["by"] = (SCALE * (w_proj @ bv) + b_proj).astype(np.float32)
        in_maps.append(im)

    nc = _get_nc(with_bias)
    global _last_in_maps
    _last_in_maps = in_maps
    res = bass_utils.run_bass_kernel_spmd(nc, in_maps, core_ids=list(range(N_CORES)))

    out = np.empty((bs, seqlen, dim), dtype=np.float32)
    for c in range(N_CORES):
        b, s = divmod(c, 2)
        out[b, s * half:(s + 1) * half, :] = res.results[c]["y"]
    return out
